# revision 19
# baseline (speedup 1.0000x reference)
"""Trainium2 Bass kernel for nn_CRInstanceLoss (hard-mining triplet loss), v4.

Reference computation (B=512, NCLASS=128, K=8, margin=1, p=1/NCLASS):
  d        = pairwise Euclidean distances of x [B, NCLASS]        (B x B)
  sim      = same-class mask; anchors = rows whose class count < 4
  mask_ap  = hard positives;  mask_an = hard negatives (top-8 per column)
  t        = relu(mask * (d[:,:,None] - d[:,None,:] + 1))          (B^3)
  out      = sum(t) / (count(t > 1e-7) + 1e-7)

v4 vs v3 (baseline 32.1us):
  * squared norms come from xT on-chip: ACT Square + ones-matmuls give
    sq in BOTH layouts with bitwise-identical values (same systolic
    contraction order), killing the xall DMA bundle, the early DVE
    reduces and the 4 sq transposes on the critical path.
  * single ACT table load: a dummy Sqrt is the first ACT op, so the
    compiler loads `sqrt_and_others` (contains square/copy/relu/sqrt)
    once at t~0 instead of stalling 1.3us mid-kernel.
  * tile U omits the per-partition halfc term (row-constant shifts do
    not change per-row top-8 ranking); the threshold is corrected
    afterwards on [128,1] via a per-partition tensor_scalar (60ns)
    before the transpose. Thresholds flip through 4 separate PSUM
    tiles, so each tile's broadcast chain starts right after its max8.
  * count ops run on GpSimd, the positives mask M on ACT - both were
    DVE work in v3; DVE is the bottleneck engine.
  * inputs split across 4 DMA queues (sync/scalar/gpsimd/vector),
    xT first.

Sharding: 8 cores x 64 anchor rows (duplicated to 128 partitions for
the top-2 positives), host combines the per-core scalar partials.
"""

import numpy as np

B = 512
NCLASS = 128
MARGIN = 1.0
BOUNDARY = 4.0   # int(B / NCLASS)
MASKC = 64.0     # additive mask unit; dominates all live values
C2 = 1024.0      # U-space offset: U = (C2 - d^2)/2 > 0 for valid pairs
DELTA_SEL = 2e-3  # threshold skew: >> ulp/add noise, << rank-8 gap ~4
EPS_CNT = 1e-7
N_CORES = 8
ROWS_PER_CORE = B // N_CORES  # 64

_CACHE = {}

# br layout (fp32r):  xT | xrdT | ones (row 0 only)
O_XT, O_XRDT, O_ONESR, BR_F = 0, 512, 640, 768
# bm1 (bf16): ne4 -- ne4[p, 512t+i] = (tgt[128t+p] != tgt[i])
BM1_F = 2048
# bm2 (bf16): nedup | simdup
O_NEDUP, O_SIMDUP, BM2_F = 0, 512, 1024
# b32 (fp32): ident
B32_F = 128


def _build():
    import concourse.bass as bass
    import concourse.bacc as bacc
    import concourse.tile as tile
    from concourse import mybir

    f32 = mybir.dt.float32
    f32r = mybir.dt.float32r
    bf16 = mybir.dt.bfloat16
    Alu = mybir.AluOpType
    Act = mybir.ActivationFunctionType
    AX = mybir.AxisListType

    nc = bacc.Bacc("TRN2", target_bir_lowering=False, debug=False,
                   num_devices=N_CORES)

    br_d = nc.dram_tensor("br", [128, BR_F], f32r, kind="ExternalInput").ap()
    bm1_d = nc.dram_tensor("bm1", [128, BM1_F], bf16, kind="ExternalInput").ap()
    bm2_d = nc.dram_tensor("bm2", [128, BM2_F], bf16, kind="ExternalInput").ap()
    b32_d = nc.dram_tensor("b32", [128, B32_F], f32, kind="ExternalInput").ap()
    out_d = nc.dram_tensor("out", [1, 4], f32, kind="ExternalOutput").ap()

    with tile.TileContext(nc) as tc:
        import contextlib
        ctx = contextlib.ExitStack()
        with ctx:
            sb = ctx.enter_context(tc.tile_pool(name="sb", bufs=1))
            scr = ctx.enter_context(tc.tile_pool(name="scr", bufs=2))
            jnk = ctx.enter_context(tc.tile_pool(name="jnk", bufs=2))
            pssel = ctx.enter_context(tc.tile_pool(name="pssel", bufs=2, space="PSUM"))
            psdup_pool = ctx.enter_context(tc.tile_pool(name="psdup", bufs=1, space="PSUM"))
            psq = ctx.enter_context(tc.tile_pool(name="psq", bufs=1, space="PSUM"))
            psh = ctx.enter_context(tc.tile_pool(name="psh", bufs=1, space="PSUM"))
            psrow = ctx.enter_context(tc.tile_pool(name="psrow", bufs=2, space="PSUM"))
            psfin = ctx.enter_context(tc.tile_pool(name="psfin", bufs=1, space="PSUM"))

            # ---------- input DMAs: 4 queues, xT first ----------
            br = sb.tile([128, BR_F], f32r)
            nc.sync.dma_start(br, br_d)
            bm1 = sb.tile([128, BM1_F], bf16)
            nc.scalar.dma_start(bm1, bm1_d)
            bm2 = sb.tile([128, BM2_F], bf16)
            nc.sync.dma_start(bm2, bm2_d)
            b32 = sb.tile([128, B32_F], f32)
            nc.gpsimd.dma_start(b32, b32_d)

            xT = br[:, O_XT:O_XT + 512]
            xrdT = br[:, O_XRDT:O_XRDT + 128]
            onesr_row = br[0:1, O_ONESR:O_ONESR + 128]
            ne_dup = bm2[:, O_NEDUP:O_NEDUP + 512]
            sim_dup = bm2[:, O_SIMDUP:O_SIMDUP + 512]
            ident = b32[:, 0:128]

            ones32 = sb.tile([128, 1], f32)
            nc.vector.memset(ones32, 1.0)
            ones32_4 = sb.tile([128, 4], f32)
            nc.vector.memset(ones32_4, 1.0)

            # dummy Sqrt first: pulls the single sqrt_and_others ACT
            # table load (covers square/copy/relu/sqrt) to t~0
            junk1 = sb.tile([128, 1], f32)
            nc.scalar.activation(junk1, ones32, Act.Sqrt)

            # ---------- squared norms from xT (both layouts) ----------
            # row layout [1,512] and column layout [128,4] come from the
            # same xTsq via PE contractions in identical d-order, so the
            # pre-round values match bitwise.
            xTsq = jnk.tile([128, 512], f32, tag="xTsq")
            nc.scalar.activation(xTsq, xT, Act.Square)
            sqrow_ps = psq.tile([1, 512], f32, tag="sqrow")
            nc.tensor.matmul(sqrow_ps, lhsT=ones32, rhs=xTsq,
                             start=True, stop=True)
            # dup-row norms from xrdT (same fp32 2-pass contraction in the
            # same d-order -> bitwise equal to the per-tile values for the
            # same rows). N=4 (all columns equal) to satisfy ISA minimums.
            xsqd = jnk.tile([128, 128], f32, tag="xsqd")
            nc.scalar.activation(xsqd, xrdT, Act.Square)
            hps = psh.tile([128, 20], f32, tag="hps")
            for t in range(4):
                nc.tensor.matmul(hps[:, 4 * t:4 * t + 4],
                                 lhsT=xTsq[:, t * 128:(t + 1) * 128],
                                 rhs=ones32_4, start=True, stop=True)
            nc.tensor.matmul(hps[:, 16:20], lhsT=xsqd, rhs=ones32_4,
                             start=True, stop=True)

            # f32r-round -sq/2 + C2/2 once; fp32-widen for scalar use.
            sqrm_off = sb.tile([1, 512], f32r)   # rank-1 rhs (free side)
            nc.scalar.activation(sqrm_off, sqrow_ps, Act.Copy, scale=-0.5,
                                 bias=C2 / 2)
            halfc_r = sb.tile([128, 20], f32r)
            nc.scalar.activation(halfc_r, hps, Act.Copy, scale=-0.5,
                                 bias=C2 / 2)
            halfc_a = sb.tile([128, 20], f32)
            nc.scalar.activation(halfc_a, halfc_r, Act.Copy)
            halfc_dup = halfc_a[:, 16:17]
            # thr correction: thr_sym+C2 = mx8 + halfc - DELTA
            hcorr = sb.tile([128, 20], f32)
            nc.vector.tensor_scalar(out=hcorr, in0=halfc_a, scalar1=-DELTA_SEL,
                                    scalar2=None, op0=Alu.add)
            bias_d2 = sb.tile([128, 1], f32)   # sq_p + C2 = -2*halfc + 2*C2
            nc.vector.tensor_scalar(out=bias_d2, in0=halfc_dup, scalar1=-2.0,
                                    scalar2=2.0 * C2, op0=Alu.mult, op1=Alu.add)

            # ---------- dup-layout chain ----------
            ps_dup = psdup_pool.tile([128, B], f32, tag="psdup")
            nc.tensor.matmul(ps_dup, lhsT=xrdT, rhs=xT, start=True, stop=False)
            nc.tensor.matmul(ps_dup, lhsT=onesr_row, rhs=sqrm_off,
                             start=False, stop=True)
            rl_dup = sb.tile([128, B], f32)  # relu(d^2): NaN-safe diagonal
            nc.scalar.activation(rl_dup, ps_dup, Act.Relu, bias=bias_d2,
                                 scale=-2.0)
            d_dup = sb.tile([128, B], f32)
            nc.scalar.activation(d_dup, rl_dup, Act.Sqrt)
            u_dup = sb.tile([128, B], f32)   # (dot + sqrm_k + halfc_p)*ne
            nc.vector.scalar_tensor_tensor(out=u_dup, in0=ps_dup,
                                           scalar=halfc_dup, in1=ne_dup,
                                           op0=Alu.add, op1=Alu.mult)

            # ---------- anchors (class counts via ACT accum) ----------
            junkS = jnk.tile([128, B], f32, tag="junkS")
            rowsum = sb.tile([128, 1], f32)
            nc.scalar.activation(junkS, sim_dup, Act.Copy, accum_out=rowsum)
            anch01 = sb.tile([128, 1], f32)
            nc.vector.tensor_scalar(out=anch01, in0=rowsum, scalar1=BOUNDARY,
                                    scalar2=None, op0=Alu.is_lt)
            anchm128 = sb.tile([128, 1], f32)  # 64*anch - 128
            nc.vector.tensor_scalar(out=anchm128, in0=anch01, scalar1=MASKC,
                                    scalar2=-2.0 * MASKC, op0=Alu.mult,
                                    op1=Alu.add)

            # ---------- positives: A = d + margin + (64*sim + anchm128) --
            M = sb.tile([128, B], f32)
            nc.scalar.activation(M, sim_dup, Act.Identity, scale=MASKC,
                                 bias=anchm128)
            A = sb.tile([128, B], f32)
            nc.vector.scalar_tensor_tensor(out=A, in0=d_dup, scalar=MARGIN,
                                           in1=M, op0=Alu.add, op1=Alu.add)
            mxA = sb.tile([128, 8], f32)
            nc.vector.max(mxA, A)
            bias_T = sb.tile([128, 1], f32)
            nc.vector.tensor_scalar(out=bias_T[0:64], in0=mxA[0:64, 0:1],
                                    scalar1=anchm128[0:64], scalar2=None,
                                    op0=Alu.add)
            nc.vector.tensor_scalar(out=bias_T[64:128], in0=mxA[64:128, 1:2],
                                    scalar1=anchm128[64:128], scalar2=None,
                                    op0=Alu.add)

            # ---------- selection tiles + fused triplet pass ----------
            s_cols = sb.tile([128, 4], f32)
            g_cols = sb.tile([128, 4], f32)
            T = sb.tile([128, B], f32)
            lt_dup = sb.tile([128, B], f32)
            negB = sb.tile([128, B], f32)
            junkT = jnk.tile([128, B], f32, tag="junkT")
            for t in range(4):
                sl = slice(t * 128, (t + 1) * 128)
                ne_t = bm1[:, t * 512:(t + 1) * 512]
                ps_d = pssel.tile([128, B], f32, tag="psd")
                nc.tensor.matmul(ps_d, lhsT=xT[:, sl], rhs=xT,
                                 start=True, stop=False)
                nc.tensor.matmul(ps_d, lhsT=onesr_row, rhs=sqrm_off,
                                 start=False, stop=True)
                u_t = scr.tile([128, B], f32, tag="u")
                nc.vector.tensor_tensor(out=u_t, in0=ps_d, in1=ne_t,
                                        op=Alu.mult)
                mx_t = sb.tile([128, 8], f32, tag=f"mx{t}", name=f"mx{t}")
                nc.vector.max(mx_t, u_t)
                mx8c = sb.tile([128, 1], f32, tag=f"mx8c{t}", name=f"mx8c{t}")
                nc.vector.tensor_scalar(out=mx8c, in0=mx_t[:, 7:8],
                                        scalar1=hcorr[:, 4 * t:4 * t + 1],
                                        scalar2=None, op0=Alu.add)
                thrrow = psrow.tile([1, 128], f32, tag="thr")
                nc.tensor.transpose(thrrow, mx8c, ident)
                m8row = sb.tile([1, 128], f32, tag=f"m8r{t}", name=f"m8r{t}")
                nc.scalar.activation(m8row, thrrow, Act.Copy)
                m8b = sb.tile([128, 128], f32, tag=f"m8b{t}", name=f"m8b{t}")
                nc.gpsimd.partition_broadcast(m8b, m8row)
                # triplet pass for this tile's 128 k-columns
                nc.vector.tensor_tensor(out=lt_dup[:, sl], in0=u_dup[:, sl],
                                        in1=m8b, op=Alu.is_ge)
                nc.vector.scalar_tensor_tensor(out=negB[:, sl],
                                               in0=lt_dup[:, sl],
                                               scalar=MASKC,
                                               in1=d_dup[:, sl],
                                               op0=Alu.mult, op1=Alu.subtract)
                nc.scalar.activation(T[:, sl], negB[:, sl], Act.Relu,
                                     bias=bias_T, scale=1.0,
                                     accum_out=s_cols[:, t:t + 1])
                nc.vector.tensor_scalar(out=junkT[:, sl], in0=T[:, sl],
                                        scalar1=EPS_CNT, scalar2=None,
                                        op0=Alu.is_gt, op1=Alu.add,
                                        accum_out=g_cols[:, t:t + 1])

            # ---------- final reductions ----------
            sg_ps = psfin.tile([1, 8], f32, tag="fin")
            nc.tensor.matmul(sg_ps[:, 0:4], lhsT=ones32, rhs=s_cols,
                             start=True, stop=True)
            nc.tensor.matmul(sg_ps[:, 4:8], lhsT=ones32, rhs=g_cols,
                             start=True, stop=True)
            fin = sb.tile([1, 4], f32)
            nc.vector.memset(fin, 0.0)
            nc.vector.reduce_sum(fin[:, 2:3], sg_ps[:, 0:4], axis=AX.X)
            nc.vector.reduce_sum(fin[:, 1:2], sg_ps[:, 4:8], axis=AX.X)
            nc.sync.dma_start(out_d, fin)

    nc.compile()
    return nc


def _host_inputs(x, target):
    """Per-core input maps: layout transforms (transpose/slice/one-hot masks)."""
    import ml_dtypes
    bf = ml_dtypes.bfloat16
    x = np.ascontiguousarray(np.asarray(x, dtype=np.float32))
    tgt = np.asarray(target).astype(np.int32).reshape(B)

    xT = np.ascontiguousarray(x.T)
    ident = np.eye(128, dtype=np.float32)

    neq = tgt[:, None] != tgt[None, :]
    # ne4[p, 512t + i] = (tgt[128t+p] != tgt[i])
    bm1 = np.ascontiguousarray(
        neq.reshape(4, 128, B).transpose(1, 0, 2).reshape(128, 2048).astype(bf))

    in_maps = []
    for c in range(N_CORES):
        r0 = c * ROWS_PER_CORE
        rows = slice(r0, r0 + ROWS_PER_CORE)
        xrd = np.vstack([x[rows], x[rows]])
        rowsel = np.concatenate([np.arange(r0, r0 + 64)] * 2)

        brr = np.zeros((128, BR_F), np.float32)
        brr[:, O_XT:O_XT + 512] = xT
        brr[:, O_XRDT:O_XRDT + 128] = xrd.T
        brr[0, O_ONESR:O_ONESR + 128] = 1.0
        bm2 = np.empty((128, BM2_F), bf)
        bm2[:, O_NEDUP:O_NEDUP + 512] = neq[rowsel].astype(bf)
        bm2[:, O_SIMDUP:O_SIMDUP + 512] = (~neq[rowsel]).astype(bf)
        in_maps.append({
            "br": np.ascontiguousarray(brr),
            "bm1": bm1,
            "bm2": np.ascontiguousarray(bm2),
            "b32": np.ascontiguousarray(ident),
        })
    return in_maps


def kernel(x, target, _trace=False):
    from concourse import bass_utils

    key = "nc"
    if key not in _CACHE:
        _CACHE[key] = _build()
    nc = _CACHE[key]
    in_maps = _host_inputs(x, target)
    res = bass_utils.run_bass_kernel_spmd(
        nc, in_maps, core_ids=list(range(N_CORES)), trace=_trace,
    )
    S = 0.0
    G = 0.0
    for rr in res.results:
        f = rr["out"].reshape(-1)
        S += float(f[2])
        G += float(f[1])
    out = np.float32(S / (G + 1e-7))
    if _trace:
        return out, res
    return out


if __name__ == "__main__":
    rng = np.random.default_rng(0)
    x = rng.standard_normal((B, NCLASS), dtype=np.float32)
    t = rng.integers(0, NCLASS, B).astype(np.int64)
    print(kernel(x, t))


# revision 24
# speedup vs baseline: 1.1744x; 1.1744x over previous
"""Trainium2 Bass kernel for nn_CRInstanceLoss (hard-mining triplet loss), v4.

Reference computation (B=512, NCLASS=128, K=8, margin=1, p=1/NCLASS):
  d        = pairwise Euclidean distances of x [B, NCLASS]        (B x B)
  sim      = same-class mask; anchors = rows whose class count < 4
  mask_ap  = hard positives;  mask_an = hard negatives (top-8 per column)
  t        = relu(mask * (d[:,:,None] - d[:,None,:] + 1))          (B^3)
  out      = sum(t) / (count(t > 1e-7) + 1e-7)

v4 vs v3 (baseline 32.1us):
  * squared norms come from xT on-chip: ACT Square + ones-matmuls give
    sq in BOTH layouts with bitwise-identical values (same systolic
    contraction order), killing the xall DMA bundle, the early DVE
    reduces and the 4 sq transposes on the critical path.
  * single ACT table load: a dummy Sqrt is the first ACT op, so the
    compiler loads `sqrt_and_others` (contains square/copy/relu/sqrt)
    once at t~0 instead of stalling 1.3us mid-kernel.
  * tile U omits the per-partition halfc term (row-constant shifts do
    not change per-row top-8 ranking); the threshold is corrected
    afterwards on [128,1] via a per-partition tensor_scalar (60ns)
    before the transpose. Thresholds flip through 4 separate PSUM
    tiles, so each tile's broadcast chain starts right after its max8.
  * count ops run on GpSimd, the positives mask M on ACT - both were
    DVE work in v3; DVE is the bottleneck engine.
  * inputs split across 4 DMA queues (sync/scalar/gpsimd/vector),
    xT first.

Sharding: 8 cores x 64 anchor rows (duplicated to 128 partitions for
the top-2 positives), host combines the per-core scalar partials.
"""

import numpy as np

B = 512
NCLASS = 128
MARGIN = 1.0
BOUNDARY = 4.0   # int(B / NCLASS)
MASKC = 64.0     # additive mask unit; dominates all live values
C2 = 1024.0      # U-space offset: U = (C2 - d^2)/2 > 0 for valid pairs
DELTA_SEL = 2e-3  # threshold skew: >> ulp/add noise, << rank-8 gap ~4
EPS_CNT = 1e-7
N_CORES = 8
ROWS_PER_CORE = B // N_CORES  # 64

_CACHE = {}

# br layout (fp32r):  xT | xrdT | ones (row 0 only) | ones cols
O_XT, O_XRDT, O_ONESR, O_ONESC, BR_F = 0, 512, 640, 768, 772
# bm1 (bf16): ne4 -- ne4[p, 512t+i] = (tgt[128t+p] != tgt[i])
BM1_F = 2048
# bm2 (bf16): nedup | simdup
O_NEDUP, O_SIMDUP, BM2_F = 0, 512, 1024
# b32 (fp32): ident
B32_F = 128


def _build():
    import concourse.bass as bass
    import concourse.bacc as bacc
    import concourse.tile as tile
    from concourse import mybir

    f32 = mybir.dt.float32
    f32r = mybir.dt.float32r
    bf16 = mybir.dt.bfloat16
    Alu = mybir.AluOpType
    Act = mybir.ActivationFunctionType
    AX = mybir.AxisListType

    nc = bacc.Bacc("TRN2", target_bir_lowering=False, debug=False,
                   num_devices=N_CORES)

    br_d = nc.dram_tensor("br", [128, BR_F], f32r, kind="ExternalInput").ap()
    bm1_d = nc.dram_tensor("bm1", [128, BM1_F], bf16, kind="ExternalInput").ap()
    bm2_d = nc.dram_tensor("bm2", [128, BM2_F], bf16, kind="ExternalInput").ap()
    b32_d = nc.dram_tensor("b32", [128, B32_F], f32, kind="ExternalInput").ap()
    out_d = nc.dram_tensor("out", [1, 4], f32, kind="ExternalOutput").ap()

    with tile.TileContext(nc) as tc:
        import contextlib
        ctx = contextlib.ExitStack()
        with ctx:
            sb = ctx.enter_context(tc.tile_pool(name="sb", bufs=1))
            scr = ctx.enter_context(tc.tile_pool(name="scr", bufs=2))
            jnk = ctx.enter_context(tc.tile_pool(name="jnk", bufs=2))
            pssel = ctx.enter_context(tc.tile_pool(name="pssel", bufs=2, space="PSUM"))
            psdup_pool = ctx.enter_context(tc.tile_pool(name="psdup", bufs=1, space="PSUM"))
            psq = ctx.enter_context(tc.tile_pool(name="psq", bufs=1, space="PSUM"))
            psh = ctx.enter_context(tc.tile_pool(name="psh", bufs=1, space="PSUM"))
            psrow = ctx.enter_context(tc.tile_pool(name="psrow", bufs=2, space="PSUM"))
            psfin = ctx.enter_context(tc.tile_pool(name="psfin", bufs=1, space="PSUM"))

            # ---------- input DMAs: 4 queues, xT first ----------
            br = sb.tile([128, BR_F], f32r)
            nc.sync.dma_start(br, br_d)
            bm1 = sb.tile([128, BM1_F], bf16)
            nc.scalar.dma_start(bm1, bm1_d)
            bm2 = sb.tile([128, BM2_F], bf16)
            nc.sync.dma_start(bm2, bm2_d)
            b32 = sb.tile([128, B32_F], f32)
            nc.gpsimd.dma_start(b32, b32_d)

            xT = br[:, O_XT:O_XT + 512]
            xrdT = br[:, O_XRDT:O_XRDT + 128]
            onesr_row = br[0:1, O_ONESR:O_ONESR + 128]
            ones_colr = br[:, O_ONESC:O_ONESC + 1]
            ones_r4 = br[:, O_ONESC:O_ONESC + 4]
            ne_dup = bm2[:, O_NEDUP:O_NEDUP + 512]
            sim_dup = bm2[:, O_SIMDUP:O_SIMDUP + 512]
            ident = b32[:, 0:128]

            ones32 = sb.tile([128, 1], f32)
            nc.vector.memset(ones32, 1.0)

            # dummy Sqrt first: pulls the single sqrt_and_others ACT
            # table load (covers square/copy/relu/sqrt) to t~0
            junk1 = sb.tile([128, 1], f32)
            nc.scalar.activation(junk1, ones32, Act.Sqrt)

            # ---------- squared norms from xT (both layouts) ----------
            # row layout [1,512] and column layout [128,4] come from the
            # same xTsq via PE contractions in identical d-order, so the
            # pre-round values match bitwise.
            xTsq = jnk.tile([128, 512], f32r, tag="xTsq")
            nc.scalar.activation(xTsq, xT, Act.Square)
            sqrow_ps = psq.tile([1, 512], f32, tag="sqrow")
            nc.tensor.matmul(sqrow_ps, lhsT=ones_colr, rhs=xTsq,
                             start=True, stop=True)
            # dup-row norms from xrdT: f32r single-pass 1.0*v products in
            # the same systolic d-order -> bitwise equal to sqrow for the
            # same rows. N=4 (all columns equal) to satisfy ISA minimums.
            xsqd = jnk.tile([128, 128], f32r, tag="xsqd")
            nc.scalar.activation(xsqd, xrdT, Act.Square)
            hps = psh.tile([128, 20], f32, tag="hps")
            for t in range(4):
                nc.tensor.matmul(hps[:, 4 * t:4 * t + 4],
                                 lhsT=xTsq[:, t * 128:(t + 1) * 128],
                                 rhs=ones_r4, start=True, stop=True)
            nc.tensor.matmul(hps[:, 16:20], lhsT=xsqd, rhs=ones_r4,
                             start=True, stop=True)

            # f32r-round -sq/2 + C2/2 once; fp32-widen for scalar use.
            sqrm_off = sb.tile([1, 512], f32r)   # rank-1 rhs (free side)
            nc.scalar.activation(sqrm_off, sqrow_ps, Act.Copy, scale=-0.5,
                                 bias=C2 / 2)
            halfc_r = sb.tile([128, 20], f32r)
            nc.scalar.activation(halfc_r, hps, Act.Copy, scale=-0.5,
                                 bias=C2 / 2)
            halfc_a = sb.tile([128, 20], f32)
            nc.scalar.activation(halfc_a, halfc_r, Act.Copy)
            halfc_dup = halfc_a[:, 16:17]
            # thr correction: thr_sym+C2 = mx8 + halfc - DELTA
            hcorr = sb.tile([128, 20], f32)
            nc.vector.tensor_scalar(out=hcorr, in0=halfc_a, scalar1=-DELTA_SEL,
                                    scalar2=None, op0=Alu.add)
            bias_d2 = sb.tile([128, 1], f32)   # sq_p + C2 = -2*halfc + 2*C2
            nc.vector.tensor_scalar(out=bias_d2, in0=halfc_dup, scalar1=-2.0,
                                    scalar2=2.0 * C2, op0=Alu.mult, op1=Alu.add)

            # ---------- dup-layout chain ----------
            ps_dup = psdup_pool.tile([128, B], f32, tag="psdup")
            nc.tensor.matmul(ps_dup, lhsT=xrdT, rhs=xT, start=True, stop=False)
            nc.tensor.matmul(ps_dup, lhsT=onesr_row, rhs=sqrm_off,
                             start=False, stop=True)
            rl_dup = sb.tile([128, B], f32)  # relu(d^2): NaN-safe diagonal
            nc.scalar.activation(rl_dup, ps_dup, Act.Relu, bias=bias_d2,
                                 scale=-2.0)
            d_dup = sb.tile([128, B], f32)
            nc.scalar.activation(d_dup, rl_dup, Act.Sqrt)
            u_dup = sb.tile([128, B], f32)   # (dot + sqrm_k + halfc_p)*ne
            nc.vector.scalar_tensor_tensor(out=u_dup, in0=ps_dup,
                                           scalar=halfc_dup, in1=ne_dup,
                                           op0=Alu.add, op1=Alu.mult)

            # ---------- anchors (class counts via ACT accum) ----------
            junkS = jnk.tile([128, B], f32, tag="junkS")
            rowsum = sb.tile([128, 1], f32)
            nc.scalar.activation(junkS, sim_dup, Act.Copy, accum_out=rowsum)
            anch01 = sb.tile([128, 1], f32)
            nc.vector.tensor_scalar(out=anch01, in0=rowsum, scalar1=BOUNDARY,
                                    scalar2=None, op0=Alu.is_lt)
            anchm128 = sb.tile([128, 1], f32)  # 64*anch - 128
            nc.vector.tensor_scalar(out=anchm128, in0=anch01, scalar1=MASKC,
                                    scalar2=-2.0 * MASKC, op0=Alu.mult,
                                    op1=Alu.add)

            # ---------- positives: A = d + margin + (64*sim + anchm128) --
            M = sb.tile([128, B], f32)
            nc.scalar.activation(M, sim_dup, Act.Identity, scale=MASKC,
                                 bias=anchm128)
            A = sb.tile([128, B], f32)
            nc.vector.scalar_tensor_tensor(out=A, in0=d_dup, scalar=MARGIN,
                                           in1=M, op0=Alu.add, op1=Alu.add)
            mxA = sb.tile([128, 8], f32)
            nc.vector.max(mxA, A)
            bias_T = sb.tile([128, 1], f32)
            nc.vector.tensor_scalar(out=bias_T[0:64], in0=mxA[0:64, 0:1],
                                    scalar1=anchm128[0:64], scalar2=None,
                                    op0=Alu.add)
            nc.vector.tensor_scalar(out=bias_T[64:128], in0=mxA[64:128, 1:2],
                                    scalar1=anchm128[64:128], scalar2=None,
                                    op0=Alu.add)

            # ---------- selection tiles + fused triplet pass ----------
            s_cols = sb.tile([128, 4], f32)
            g_cols = sb.tile([128, 4], f32)
            T = sb.tile([128, B], f32)
            lt_dup = sb.tile([128, B], f32)
            negB = sb.tile([128, B], f32)
            junkT = jnk.tile([128, B], f32, tag="junkT")
            for t in range(4):
                sl = slice(t * 128, (t + 1) * 128)
                ne_t = bm1[:, t * 512:(t + 1) * 512]
                ps_d = pssel.tile([128, B], f32, tag="psd")
                nc.tensor.matmul(ps_d, lhsT=xT[:, sl], rhs=xT,
                                 start=True, stop=False)
                nc.tensor.matmul(ps_d, lhsT=onesr_row, rhs=sqrm_off,
                                 start=False, stop=True)
                u_t = scr.tile([128, B], f32, tag="u")
                nc.vector.tensor_tensor(out=u_t, in0=ps_d, in1=ne_t,
                                        op=Alu.mult)
                mx_t = sb.tile([128, 8], f32, tag=f"mx{t}", name=f"mx{t}")
                nc.vector.max(mx_t, u_t)
                mx8c = sb.tile([128, 1], f32, tag=f"mx8c{t}", name=f"mx8c{t}")
                nc.vector.tensor_scalar(out=mx8c, in0=mx_t[:, 7:8],
                                        scalar1=hcorr[:, 4 * t:4 * t + 1],
                                        scalar2=None, op0=Alu.add)
                thrrow = psrow.tile([1, 128], f32, tag="thr")
                nc.tensor.transpose(thrrow, mx8c, ident)
                m8row = sb.tile([1, 128], f32, tag=f"m8r{t}", name=f"m8r{t}")
                nc.scalar.activation(m8row, thrrow, Act.Copy)
                m8b = sb.tile([128, 128], f32, tag=f"m8b{t}", name=f"m8b{t}")
                nc.gpsimd.partition_broadcast(m8b, m8row)
                # triplet pass for this tile's 128 k-columns
                nc.vector.tensor_tensor(out=lt_dup[:, sl], in0=u_dup[:, sl],
                                        in1=m8b, op=Alu.is_ge)
                nc.vector.scalar_tensor_tensor(out=negB[:, sl],
                                               in0=lt_dup[:, sl],
                                               scalar=MASKC,
                                               in1=d_dup[:, sl],
                                               op0=Alu.mult, op1=Alu.subtract)
                nc.scalar.activation(T[:, sl], negB[:, sl], Act.Relu,
                                     bias=bias_T, scale=1.0,
                                     accum_out=s_cols[:, t:t + 1])
                nc.vector.tensor_scalar(out=junkT[:, sl], in0=T[:, sl],
                                        scalar1=EPS_CNT, scalar2=None,
                                        op0=Alu.is_gt, op1=Alu.add,
                                        accum_out=g_cols[:, t:t + 1])

            # ---------- final reductions ----------
            sg_ps = psfin.tile([1, 8], f32, tag="fin")
            nc.tensor.matmul(sg_ps[:, 0:4], lhsT=ones32, rhs=s_cols,
                             start=True, stop=True)
            nc.tensor.matmul(sg_ps[:, 4:8], lhsT=ones32, rhs=g_cols,
                             start=True, stop=True)
            fin = sb.tile([1, 4], f32)
            nc.vector.memset(fin, 0.0)
            nc.vector.reduce_sum(fin[:, 2:3], sg_ps[:, 0:4], axis=AX.X)
            nc.vector.reduce_sum(fin[:, 1:2], sg_ps[:, 4:8], axis=AX.X)
            nc.sync.dma_start(out_d, fin)

    nc.compile()
    return nc


def _host_inputs(x, target):
    """Per-core input maps: layout transforms (transpose/slice/one-hot masks)."""
    import ml_dtypes
    bf = ml_dtypes.bfloat16
    x = np.ascontiguousarray(np.asarray(x, dtype=np.float32))
    tgt = np.asarray(target).astype(np.int32).reshape(B)

    xT = np.ascontiguousarray(x.T)
    ident = np.eye(128, dtype=np.float32)

    neq = tgt[:, None] != tgt[None, :]
    # ne4[p, 512t + i] = (tgt[128t+p] != tgt[i])
    bm1 = np.ascontiguousarray(
        neq.reshape(4, 128, B).transpose(1, 0, 2).reshape(128, 2048).astype(bf))

    in_maps = []
    for c in range(N_CORES):
        r0 = c * ROWS_PER_CORE
        rows = slice(r0, r0 + ROWS_PER_CORE)
        xrd = np.vstack([x[rows], x[rows]])
        rowsel = np.concatenate([np.arange(r0, r0 + 64)] * 2)

        brr = np.zeros((128, BR_F), np.float32)
        brr[:, O_XT:O_XT + 512] = xT
        brr[:, O_XRDT:O_XRDT + 128] = xrd.T
        brr[0, O_ONESR:O_ONESR + 128] = 1.0
        brr[:, O_ONESC:O_ONESC + 4] = 1.0
        bm2 = np.empty((128, BM2_F), bf)
        bm2[:, O_NEDUP:O_NEDUP + 512] = neq[rowsel].astype(bf)
        bm2[:, O_SIMDUP:O_SIMDUP + 512] = (~neq[rowsel]).astype(bf)
        in_maps.append({
            "br": np.ascontiguousarray(brr),
            "bm1": bm1,
            "bm2": np.ascontiguousarray(bm2),
            "b32": np.ascontiguousarray(ident),
        })
    return in_maps


def kernel(x, target, _trace=False):
    from concourse import bass_utils

    key = "nc"
    if key not in _CACHE:
        _CACHE[key] = _build()
    nc = _CACHE[key]
    in_maps = _host_inputs(x, target)
    res = bass_utils.run_bass_kernel_spmd(
        nc, in_maps, core_ids=list(range(N_CORES)), trace=_trace,
    )
    S = 0.0
    G = 0.0
    for rr in res.results:
        f = rr["out"].reshape(-1)
        S += float(f[2])
        G += float(f[1])
    out = np.float32(S / (G + 1e-7))
    if _trace:
        return out, res
    return out


if __name__ == "__main__":
    rng = np.random.default_rng(0)
    x = rng.standard_normal((B, NCLASS), dtype=np.float32)
    t = rng.integers(0, NCLASS, B).astype(np.int64)
    print(kernel(x, t))
